# revision 4
# baseline (speedup 1.0000x reference)
"""Trainium2 Bass kernel for nn_CausalAttention (N=4096, 8 heads, DH=32).

Strategy: head-parallel across 8 NeuronCores (1 head per core).
Per core:
  - QKV projections from channels-major inputs [256, 4096] (natural layout
    is already the transposed layout the TensorEngine wants).
  - Scores computed transposed: S^T[k, q] = K @ Q^T, in 512-query blocks,
    3 k-tiles (128 keys each) per PSUM group via row-packed K=32 matmuls.
  - Max-free softmax: P^T = exp(S / sqrt(32)) with strict-causal 0/1 mask
    applied post-exp (scores are O(1), so exp never overflows; reference's
    -10000 masking underflows to exactly 0 in f32, matching the 0-mask).
  - Softmax denominator folded into the PV matmul via a ones column
    appended to V (lhsT [128, 33]); P^T and V in bf16 (f32 accumulate).
  - Per 512-q block: O^T_unnorm [33, 512] -> PE transpose -> [128, 33],
    reciprocal of col 32 -> per-partition scale -> out [4096, 32] per core.
Host gathers the eight [4096, 32] outputs into [1, 256, 64, 64].
"""

import math

import numpy as np
import ml_dtypes

import concourse.bass as bass
import concourse.mybir as mybir
from concourse import bacc
from concourse.tile import TileContext
from concourse.bass_utils import run_bass_kernel_spmd

# Problem constants (hardcoded per harness contract).
B, CQ, CK, CH, NH, H, W = 1, 256, 256, 256, 8, 64, 64
DH = CH // NH            # 32
N = H * W                # 4096
P = 128                  # SBUF partitions
QB = 512                 # queries per block
NQB = N // QB            # 8
KT = 128                 # keys per k-tile
NKT = N // KT            # 32
GS = 3                   # k-tiles per S-group (3 PSUM banks per group)
NG = (NKT + GS - 1) // GS  # 11 column-groups in packed kT layout
SCALE = 1.0 / math.sqrt(DH)

F32 = mybir.dt.float32
F32R = mybir.dt.float32r
BF16 = mybir.dt.bfloat16

_CACHED_NC = None


def _build():
    nc = bacc.Bacc("TRN2", target_bir_lowering=False, debug=False, num_devices=1)

    qin_d = nc.dram_tensor("qin", [CQ, N], F32, kind="ExternalInput")
    kin_d = nc.dram_tensor("kin", [CK, N], F32, kind="ExternalInput")
    wq_d = nc.dram_tensor("wqt", [CQ, 128], F32, kind="ExternalInput")
    wk_d = nc.dram_tensor("wkt", [CK, 128], F32, kind="ExternalInput")
    wv_d = nc.dram_tensor("wvt", [CK, DH], F32, kind="ExternalInput")
    bq_d = nc.dram_tensor("bqr", [128, 1], F32, kind="ExternalInput")
    bk_d = nc.dram_tensor("bkr", [128, 1], F32, kind="ExternalInput")
    bv_d = nc.dram_tensor("bvr", [128, DH], F32, kind="ExternalInput")
    out_d = nc.dram_tensor("out", [N, DH], F32, kind="ExternalOutput")

    # Strict-causal mask template: tm[kk, j] = 1.0 iff kk < j - 384.
    # For a diagonal sub-tile with offset o = 128*j - 512*qb in {0,128,256,384},
    # slicing tm[:, 384-o : 512] against P^T[:, 0 : o+128] gives
    # mask[kk, qq] = (kk < qq - o)  ==  (global k < global q).
    tm_np = (np.arange(128)[:, None] < (np.arange(512)[None, :] - 384)).astype(
        ml_dtypes.bfloat16
    )
    tm_d = nc.inline_tensor(tm_np, name="tmask")
    id_d = nc.inline_tensor(np.eye(128, dtype=np.float32), name="ident")

    with TileContext(nc) as tc:
        with (
            tc.tile_pool(name="constp", bufs=1) as constp,
            tc.tile_pool(name="bigp", bufs=1) as bigp,
            tc.tile_pool(name="workp", bufs=3) as workp,
            tc.tile_pool(name="spool", bufs=2, space="PSUM") as spool,
            tc.tile_pool(name="mpool", bufs=2, space="PSUM") as mpool,
        ):
            # ---- constants / weights to SBUF ----
            tm_sb = constp.tile([128, 512], BF16, name="tm_sb")
            nc.sync.dma_start(tm_sb[:], tm_d.ap())
            id_sb = constp.tile([128, 128], F32, name="id_sb")
            nc.sync.dma_start(id_sb[:], id_d.ap())
            wq_sb = constp.tile([128, 2, 128], F32R, name="wq_sb")
            nc.sync.dma_start(
                wq_sb[:], wq_d.ap().rearrange("(c p) m -> p c m", p=128).bitcast(F32R)
            )
            wk_sb = constp.tile([128, 2, 128], F32R, name="wk_sb")
            nc.sync.dma_start(
                wk_sb[:], wk_d.ap().rearrange("(c p) m -> p c m", p=128).bitcast(F32R)
            )
            wv_sb = constp.tile([128, 2, DH], F32R, name="wv_sb")
            nc.sync.dma_start(
                wv_sb[:], wv_d.ap().rearrange("(c p) m -> p c m", p=128).bitcast(F32R)
            )
            bq_sb = constp.tile([128, 1], F32, name="bq_sb")
            nc.sync.dma_start(bq_sb[:], bq_d.ap())
            bk_sb = constp.tile([128, 1], F32, name="bk_sb")
            nc.sync.dma_start(bk_sb[:], bk_d.ap())
            bv_sb = constp.tile([128, DH], F32, name="bv_sb")
            nc.sync.dma_start(bv_sb[:], bv_d.ap())

            # ---- stream inputs (channels-major [2*128, 4096]) ----
            kin_sb = bigp.tile([128, 2, N], F32R, name="kin_sb")
            qin_sb = bigp.tile([128, 2, N], F32R, name="qin_sb")
            kin_ap = kin_d.ap().rearrange("(c p) n -> p c n", p=128).bitcast(F32R)
            qin_ap = qin_d.ap().rearrange("(c p) n -> p c n", p=128).bitcast(F32R)
            for h in range(4):
                sl = slice(1024 * h, 1024 * (h + 1))
                for ch in range(2):
                    nc.sync.dma_start(kin_sb[:, ch, sl], kin_ap[:, ch, sl])
                for ch in range(2):
                    nc.sync.dma_start(qin_sb[:, ch, sl], qin_ap[:, ch, sl])

            # ---- projections ----
            # kT3[32u+d, 128g+kk] = k^T[d, 128*(3g+u)+kk]  (3-way row packing)
            kT3 = bigp.tile([96, NG * 128], F32R, name="kT3")
            # qT[32u+d, q] = q^T[d, q] for u=0..3 (4x replicated on partitions)
            qT = bigp.tile([128, N], F32R, name="qT")
            # v_all[kk, t, :DH] = v[128t+kk, :]; col DH is the ones column
            v_all = bigp.tile([128, NKT, DH + 1], BF16, name="v_all")
            nc.vector.memset(v_all[:, :, DH : DH + 1], 1.0)

            for s in range(8):
                ksl = slice(512 * s, 512 * (s + 1))
                pj = mpool.tile([128, 512], F32, name="pj", tag="m")
                for ch in range(2):
                    nc.tensor.matmul(
                        pj[:],
                        wk_sb[:, ch, :],
                        kin_sb[:, ch, ksl],
                        start=(ch == 0),
                        stop=(ch == 1),
                    )
                for ci in range(4):
                    j = 4 * s + ci
                    u, g = j % GS, j // GS
                    nc.vector.tensor_scalar_add(
                        kT3[32 * u : 32 * u + 32, 128 * g : 128 * g + 128],
                        pj[32 * u : 32 * u + 32, 128 * ci : 128 * ci + 128],
                        bk_sb[32 * u : 32 * u + 32, :],
                    )
                pj = mpool.tile([128, 512], F32, name="pj", tag="m")
                for ch in range(2):
                    nc.tensor.matmul(
                        pj[:],
                        wq_sb[:, ch, :],
                        qin_sb[:, ch, ksl],
                        start=(ch == 0),
                        stop=(ch == 1),
                    )
                nc.vector.tensor_scalar_add(qT[:, ksl], pj[:], bq_sb[:])
                for t in range(4 * s, 4 * s + 4):
                    nsl = slice(128 * t, 128 * (t + 1))
                    pj = mpool.tile([128, DH], F32, name="pj", tag="m")
                    for ch in range(2):
                        nc.tensor.matmul(
                            pj[:],
                            kin_sb[:, ch, nsl],
                            wv_sb[:, ch, :],
                            start=(ch == 0),
                            stop=(ch == 1),
                        )
                    nc.vector.tensor_add(v_all[:, t, 0:DH], pj[:], bv_sb[:])

            # ---- attention over q-blocks ----
            for qb in range(NQB):
                qsl = slice(512 * qb, 512 * (qb + 1))
                o_ps = mpool.tile([DH + 1, 512], F32, name="o_ps", tag="m")
                nkt_q = 4 * (qb + 1)          # causal: k-tiles 0..nkt_q-1
                ngr = (nkt_q + GS - 1) // GS
                first = True
                for g in range(ngr):
                    nsub = min(GS, nkt_q - GS * g)
                    s_ps = spool.tile([128, GS * 512], F32, name="s_ps")
                    for u in range(nsub):
                        j = GS * g + u
                        o = max(0, 128 * j - 512 * qb)
                        nc.tensor.matmul(
                            s_ps[:, 512 * u + o : 512 * (u + 1)],
                            kT3[32 * u : 32 * u + 32, 128 * g : 128 * g + 128],
                            qT[32 * u : 32 * u + 32, 512 * qb + o : 512 * (qb + 1)],
                            start=True,
                            stop=True,
                        )
                    p_sb = workp.tile([128, GS * 512], BF16, name="p_sb", bufs=6)
                    nc.scalar.activation(
                        p_sb[:, 0 : 512 * nsub],
                        s_ps[:, 0 : 512 * nsub],
                        mybir.ActivationFunctionType.Exp,
                        scale=SCALE,
                    )
                    for u in range(nsub):
                        j = GS * g + u
                        o = 128 * j - 512 * qb
                        if o > 0:  # zero the fully-masked prefix (stale exp)
                            nc.vector.memset(p_sb[:, 512 * u : 512 * u + o], 0.0)
                        if o >= 0:  # strict-causal mask on the diagonal window
                            nc.vector.tensor_mul(
                                p_sb[:, 512 * u + o : 512 * u + o + 128],
                                p_sb[:, 512 * u + o : 512 * u + o + 128],
                                tm_sb[:, 384:512],
                            )
                    for u in range(nsub):
                        j = GS * g + u
                        nc.tensor.matmul(
                            o_ps[:],
                            v_all[:, j, :],
                            p_sb[:, 512 * u : 512 * (u + 1)],
                            start=first,
                            stop=(g == ngr - 1 and u == nsub - 1),
                            skip_group_check=True,
                        )
                        first = False

                # block tail: transpose, normalize, store
                ot_sb = workp.tile([DH + 1, 512], F32, name="ot_sb")
                nc.vector.tensor_copy(ot_sb[:], o_ps[:])
                for t in range(4):
                    tr_ps = mpool.tile([128, DH + 1], F32, name="tr_ps", tag="m")
                    nc.tensor.transpose(
                        tr_ps[:],
                        ot_sb[:, 128 * t : 128 * (t + 1)],
                        id_sb[0 : DH + 1, 0 : DH + 1],
                    )
                    rinv = workp.tile([128, 1], F32, name="rinv")
                    # +1e-30 keeps q=0 (fully masked row) at 0 instead of NaN
                    nc.vector.tensor_scalar_add(
                        rinv[:], tr_ps[:, DH : DH + 1], 1e-30
                    )
                    nc.vector.reciprocal(rinv[:], rinv[:])
                    oo = workp.tile([128, DH], F32, name="oo")
                    nc.vector.tensor_scalar_mul(oo[:], tr_ps[:, 0:DH], rinv[:])
                    r0 = 512 * qb + 128 * t
                    nc.sync.dma_start(out_d.ap()[r0 : r0 + 128, :], oo[:])

    nc.finalize()
    return nc


def _get_nc():
    global _CACHED_NC
    if _CACHED_NC is None:
        _CACHED_NC = _build()
    return _CACHED_NC


def _prep_in_maps(inputs):
    f = lambda a: np.ascontiguousarray(np.asarray(a, dtype=np.float32))
    query = f(inputs["query"]).reshape(CQ, N)
    key_feat = f(inputs["key_feat"]).reshape(CK, N)

    def wnorm(v, g):
        v = f(v)
        g = f(g)
        return g[:, None] * v / np.linalg.norm(v, axis=1, keepdims=True)

    wq = wnorm(inputs["vq"], inputs["gq"])
    wk = wnorm(inputs["vk"], inputs["gk"])
    wv = wnorm(inputs["vv"], inputs["gv"])
    bq, bk, bv = f(inputs["bq"]), f(inputs["bk"]), f(inputs["bv"])

    in_maps = []
    for c in range(NH):
        rows = slice(DH * c, DH * (c + 1))
        in_maps.append(
            {
                "qin": query,
                "kin": key_feat,
                "wqt": np.ascontiguousarray(np.tile(wq[rows].T, (1, 4))),
                "wkt": np.ascontiguousarray(np.tile(wk[rows].T, (1, 4))),
                "wvt": np.ascontiguousarray(wv[rows].T),
                "bqr": np.ascontiguousarray(np.tile(bq[rows], 4)[:, None]),
                "bkr": np.ascontiguousarray(np.tile(bk[rows], 4)[:, None]),
                "bvr": np.ascontiguousarray(np.tile(bv[rows][None, :], (128, 1))),
            }
        )
    return in_maps


def _run(inputs, trace=False, **kwargs):
    nc = _get_nc()
    in_maps = _prep_in_maps(inputs)
    res = run_bass_kernel_spmd(
        nc, in_maps, core_ids=list(range(NH)), trace=trace, **kwargs
    )
    out = np.empty((B, CH, H, W), dtype=np.float32)
    for c in range(NH):
        oc = res.results[c]["out"]  # [N, DH]
        out[0, DH * c : DH * (c + 1)] = oc.T.reshape(DH, H, W)
    return out, res


def kernel(**inputs) -> np.ndarray:
    out, _ = _run(inputs, trace=False)
    return out
